# revision 1
# baseline (speedup 1.0000x reference)
"""Trainium2 Bass kernel for the DeepKalmanFilter problem.

Sharding: data-parallel over batch (2048 -> 8 cores x 256), weights replicated.

Device layout is feature-major [features, batch] for the two sequential scans
(backward LSTM + DKF inference), with a packed [128, T*B/8] layout for the
bulk decoder/transition MLPs (8 timesteps stacked into the partition dim via
block-diagonal weights).  Outputs are written in packed device layouts and
unpacked on the host.
"""

import sys

sys.path.insert(0, "/opt/trn_rl_repo")

import numpy as np

X_DIM, Z_DIM, H_DIM, G_DIM, I_DIM = 1, 16, 16, 8, 16
T_LEN, BATCH, N_CORES = 512, 2048, 8
BP = BATCH // N_CORES  # batch per core = 256

# matmul dtype: 'f32' (safe), 'f32r' (fast fp32 mode), 'bf16'
MM_DTYPE = "f32"

_CACHE = {}


def _f(x):
    return np.ascontiguousarray(x, dtype=np.float32)


def _prep_weights(inp):
    """Host-side packing of all weight tensors into matmul-ready layouts.

    Fusions are computed in float64 then cast, to stay close to the fp32
    reference.
    """
    W = {}
    # ---- LSTM: gate groups placed at aligned partition bases of the psum
    # output: i@0:16, f@32:48, o@64:80, g@96:112 (pad cols are zero ->
    # sigmoid(0)/tanh(0) garbage rows, never read) ----
    Wih = np.asarray(inp["lstm_Wih"])  # (64, 1)
    Whh = np.asarray(inp["lstm_Whh"])  # (64, 16)
    bih = np.asarray(inp["lstm_bih"])
    bhh = np.asarray(inp["lstm_bhh"])
    # original gate order i,f,g,o in rows of Wih/Whh
    gsrc = {"i": slice(0, 16), "f": slice(16, 32), "g": slice(32, 48), "o": slice(48, 64)}
    gdst = {"i": 0, "f": 32, "o": 64, "g": 96}
    # rhs layout: rows 0:16 h, row 16 x, rows 17:32 pad, row 32 ones
    lstm_lhsT = np.zeros((33, 112), np.float64)
    bsum = bih.astype(np.float64) + bhh.astype(np.float64)
    for k in ("i", "f", "o", "g"):
        d = gdst[k]
        lstm_lhsT[0:16, d : d + 16] = Whh[gsrc[k], :].T
        lstm_lhsT[16, d : d + 16] = Wih[gsrc[k], 0]
        lstm_lhsT[32, d : d + 16] = bsum[gsrc[k]]
    W["lstm_lhsT"] = _f(lstm_lhsT)

    # ---- combiner layer 1: hz=[h; z] -> 16, bias folded ----
    # rhs layout: rows 0:16 z, rows 16:32 h, row 32 ones
    cW1 = np.asarray(inp["comb_W1"])  # (16, 32): cols 0:16 -> h, 16:32 -> z
    comb1_lhsT = np.zeros((33, 16), np.float64)
    comb1_lhsT[0:16, :] = cW1[:, 16:32].T
    comb1_lhsT[16:32, :] = cW1[:, 0:16].T
    comb1_lhsT[32, :] = np.asarray(inp["comb_b1"])
    W["comb1_lhsT"] = _f(comb1_lhsT)

    # ---- fused comb2+enc1 (linear x linear collapses): W' = enc_W1 @ comb_W2
    eW1 = np.asarray(inp["enc_W1"]).astype(np.float64)  # (16, 8)
    cW2 = np.asarray(inp["comb_W2"]).astype(np.float64)  # (8, 16)
    Wp = eW1 @ cW2  # (16, 16)
    bp = eW1 @ np.asarray(inp["comb_b2"]).astype(np.float64) + np.asarray(
        inp["enc_b1"]
    ).astype(np.float64)
    wp_lhsT = np.zeros((33, 16), np.float64)
    wp_lhsT[0:16, :] = Wp.T
    wp_lhsT[32, :] = bp
    W["wp_lhsT"] = _f(wp_lhsT)

    # ---- encoder layer 2 -> (mu_z, logvar_z) ----
    # output rows: mu@0:16, lv@32:48 (aligned partition bases)
    eW2 = np.asarray(inp["enc_W2"])  # (32, 16)
    eb2 = np.asarray(inp["enc_b2"])
    enc2_lhsT = np.zeros((33, 48), np.float64)
    enc2_lhsT[0:16, 0:16] = eW2[0:16, :].T
    enc2_lhsT[32, 0:16] = eb2[0:16]
    enc2_lhsT[0:16, 32:48] = eW2[16:32, :].T
    enc2_lhsT[32, 32:48] = eb2[16:32]
    W["enc2_lhsT"] = _f(enc2_lhsT)

    # ---- decoder (blockdiag8 packing over timestep groups) ----
    dW1 = np.asarray(inp["dec_W1"])  # (16, 16)
    dec1bd = np.zeros((128, 128), np.float64)
    for g in range(8):
        dec1bd[16 * g : 16 * g + 16, 16 * g : 16 * g + 16] = dW1.T
    W["dec1bd"] = _f(dec1bd)
    W["dec1b"] = _f(np.tile(np.asarray(inp["dec_b1"]), 8).reshape(128, 1))
    dW2 = np.asarray(inp["dec_W2"])  # (2, 16)
    dec2bd = np.zeros((128, 16), np.float64)
    for g in range(8):
        dec2bd[16 * g : 16 * g + 16, 2 * g : 2 * g + 2] = dW2.T
    W["dec2bd"] = _f(dec2bd)
    W["dec2b"] = _f(np.tile(np.asarray(inp["dec_b2"]), 8).reshape(16, 1))

    # ---- transition (shifted blockdiag: out group g reads z at group g-1) ----
    tW1 = np.asarray(inp["tr_W1"])  # (16, 16)
    tr1bd = np.zeros((128, 128), np.float64)
    for g in range(1, 8):
        tr1bd[16 * (g - 1) : 16 * g, 16 * g : 16 * g + 16] = tW1.T
    W["tr1bd"] = _f(tr1bd)
    W["tr1g0"] = _f(tW1.T)  # (16, 16) fixup for group 0 (reads prev col grp 7)
    W["tr1b"] = _f(np.tile(np.asarray(inp["tr_b1"]), 8).reshape(128, 1))
    tW2 = np.asarray(inp["tr_W2"])  # (32, 16)
    # stacked twice: rows 0:64 used for step-groups 0-3 (rhs base partition 0),
    # rows 64:128 for groups 4-7 (rhs base partition 64) -- PE requires lhsT
    # and rhs to share the same base partition.
    tr2bd = np.zeros((128, 128), np.float64)
    for h in range(4):
        tr2bd[16 * h : 16 * h + 16, 32 * h : 32 * h + 32] = tW2.T
        tr2bd[64 + 16 * h : 64 + 16 * h + 16, 32 * h : 32 * h + 32] = tW2.T
    W["tr2bd"] = _f(tr2bd)
    W["tr2b"] = _f(np.tile(np.asarray(inp["tr_b2"]), 4).reshape(128, 1))
    return W


WEIGHT_SHAPES = {
    "lstm_lhsT": (33, 112),
    "comb1_lhsT": (33, 16),
    "wp_lhsT": (33, 16),
    "enc2_lhsT": (33, 48),
    "dec1bd": (128, 128),
    "dec1b": (128, 1),
    "dec2bd": (128, 16),
    "dec2b": (16, 1),
    "tr1bd": (128, 128),
    "tr1g0": (16, 16),
    "tr1b": (128, 1),
    "tr2bd": (128, 128),
    "tr2b": (128, 1),
}

_MM_WEIGHTS = [
    "lstm_lhsT",
    "comb1_lhsT",
    "wp_lhsT",
    "enc2_lhsT",
    "dec1bd",
    "dec2bd",
    "tr1bd",
    "tr1g0",
    "tr2bd",
]


def build_nc(T=T_LEN, mm_dtype=MM_DTYPE):
    import concourse.bacc as bacc
    import concourse.tile as tile
    from concourse import mybir

    f32 = mybir.dt.float32
    mmdt = mybir.dt.bfloat16 if mm_dtype == "bf16" else mybir.dt.float32
    f32r = mybir.dt.float32r

    def _bc(ap):
        # float32r is an fp32 bit-compatible PE fast mode; applied via bitcast
        return ap.bitcast(f32r) if mm_dtype == "f32r" else ap

    AF = mybir.ActivationFunctionType

    NB = T // 8  # column blocks of zpacked/hseq
    NM = T // 16  # phase-C blocks

    nc = bacc.Bacc(None)

    x_in = nc.dram_tensor("x_r", [T, BP], mmdt, kind="ExternalInput")
    eps_in = nc.dram_tensor("eps_fm", [T, Z_DIM, BP], f32, kind="ExternalInput")
    wdram = {}
    for name, shape in WEIGHT_SHAPES.items():
        wdram[name] = nc.dram_tensor(name, list(shape), f32, kind="ExternalInput")

    out_e = nc.dram_tensor("out_e", [T, 32, BP], f32, kind="ExternalOutput")
    out_d = nc.dram_tensor("out_d", [NM, 16, 512], f32, kind="ExternalOutput")
    out_t = nc.dram_tensor("out_t", [NM, 2, 128, 512], f32, kind="ExternalOutput")

    RA = 4
    RB = 4

    with tile.TileContext(nc) as tc:
        with tc.tile_pool(name="persist", bufs=1) as pp:
            # ------------- weights + persistent state -------------
            wt = {}
            for name, shape in WEIGHT_SHAPES.items():
                wt[name] = pp.tile(list(shape), f32, name=f"w_{name}", tag=f"w_{name}")
                nc.sync.dma_start(out=wt[name][:, :], in_=wdram[name][:, :])
            if mmdt != f32:
                wtm = {}
                for name in _MM_WEIGHTS:
                    shape = WEIGHT_SHAPES[name]
                    wtm[name] = pp.tile(
                        list(shape), mmdt, name=f"wm_{name}", tag=f"wm_{name}"
                    )
                    nc.vector.tensor_copy(wtm[name][:, :], wt[name][:, :])
            else:
                wtm = wt

            hseq = pp.tile([128, NB * BP], mmdt, name="hseq", tag="hseq")
            zpk = pp.tile([128, NB * BP], mmdt, name="zpk", tag="zpk")
            zzero = pp.tile([16, BP], mmdt, name="zzero", tag="zzero")
            nc.vector.memset(zzero[:, :], 0.0)
            cst = pp.tile([16, BP], f32, name="c_state", tag="c_state")
            nc.vector.memset(cst[:, :], 0.0)

            # ======================= Phase A: backward LSTM ===================
            with (
                tc.tile_pool(name="arings", bufs=1) as arp,
                tc.tile_pool(name="apsum", bufs=2, space="PSUM") as psA,
                tc.tile_pool(name="asb", bufs=3) as sA,
            ):
                arhs = []
                for r in range(RA):
                    # rows 0:16 h, row 16 x, rows 17:32 pad, row 32 ones
                    ttile = arp.tile([33, BP], mmdt, name=f"arhs{r}", tag=f"arhs{r}")
                    nc.vector.memset(ttile[0:32, :], 0.0)
                    nc.vector.memset(ttile[32:33, :], 1.0)
                    arhs.append(ttile)
                nc.vector.memset(arhs[0][0:16, :], 0.0)
                nc.sync.dma_start(out=arhs[0][16:17, :], in_=x_in[T - 1 : T, :])

                # two independent batch chunks (columns) pipeline their
                # per-step dependency chains across the engines
                HB = BP // 2
                chs = [slice(0, HB), slice(HB, BP)]
                def a_stage1(j, ci, cs):
                    r = j % RA
                    pg = psA.tile(
                        [112, HB], f32, name=f"pg{ci}", tag=f"pg{ci}", bufs=2
                    )
                    nc.tensor.matmul(
                        pg[:, :],
                        _bc(wtm["lstm_lhsT"][:, :]),
                        _bc(arhs[r][:, cs]),
                        start=True,
                        stop=True,
                    )
                    # sact lives in PSUM: DVE tensor-tensor ops with
                    # unequal base partitions need one PSUM input
                    sact = psA.tile(
                        [80, HB], f32, name=f"sact{ci}", tag=f"sact{ci}", bufs=2
                    )
                    nc.scalar.activation(sact[:, :], pg[0:80, :], AF.Sigmoid)
                    tg = sA.tile([16, HB], f32, name=f"tg{ci}", tag=f"tg{ci}")
                    nc.scalar.activation(tg[:, :], pg[96:112, :], AF.Tanh)
                    u = sA.tile([16, HB], f32, name=f"u{ci}", tag=f"u{ci}")
                    nc.vector.tensor_mul(u[:, :], sact[32:48, :], cst[:, cs])
                    v = sA.tile([16, HB], f32, name=f"v{ci}", tag=f"v{ci}")
                    nc.vector.tensor_mul(v[:, :], sact[0:16, :], tg[:, :])
                    nc.vector.tensor_add(cst[:, cs], u[:, :], v[:, :])
                    return sact

                def a_stage2(j, ci, cs, sact):
                    rn = (j + 1) % RA
                    th = sA.tile([16, HB], f32, name=f"th{ci}", tag=f"th{ci}")
                    nc.scalar.activation(th[:, :], cst[:, cs], AF.Tanh)
                    # h -> next rhs ring slot (rows 0:16)
                    nc.vector.tensor_mul(arhs[rn][0:16, cs], sact[64:80, :], th[:, :])

                for j in range(T):
                    t = T - 1 - j
                    rn = (j + 1) % RA
                    # staged emission: both chunks' first halves enqueue before
                    # either second half, so neither chain head-of-line blocks
                    # the other on the ACT/DVE queues
                    sa0 = a_stage1(j, 0, chs[0])
                    sa1 = a_stage1(j, 1, chs[1])
                    a_stage2(j, 0, chs[0], sa0)
                    a_stage2(j, 1, chs[1], sa1)
                    if j < T - 1:
                        nc.sync.dma_start(out=arhs[rn][16:17, :], in_=x_in[t - 1 : t, :])
                    # persist h_t for the DKF combiner input
                    g, cc = t % 8, t // 8
                    hs = hseq[16 * g : 16 * g + 16, BP * cc : BP * cc + BP]
                    nc.sync.dma_start(out=hs, in_=arhs[rn][0:16, :])

            # ======================= Phase B: DKF scan ========================
            with (
                tc.tile_pool(name="brings", bufs=1) as brp,
                tc.tile_pool(name="bpsum", bufs=2, space="PSUM") as psB,
                tc.tile_pool(name="bsb", bufs=3) as sB,
            ):
                bhz, bt1, be1, beps = [], [], [], []
                for r in range(RB):
                    # rows 0:16 z, rows 16:32 h, row 32 ones
                    ttile = brp.tile([33, BP], mmdt, name=f"bhz{r}", tag=f"bhz{r}")
                    nc.vector.memset(ttile[0:32, :], 0.0)
                    nc.vector.memset(ttile[32:33, :], 1.0)
                    bhz.append(ttile)
                    # rows 0:16 payload, rows 17:32 pad, row 32 ones
                    t1 = brp.tile([33, BP], mmdt, name=f"bt1{r}", tag=f"bt1{r}")
                    nc.vector.memset(t1[0:32, :], 0.0)
                    nc.vector.memset(t1[32:33, :], 1.0)
                    bt1.append(t1)
                    e1 = brp.tile([33, BP], mmdt, name=f"be1{r}", tag=f"be1{r}")
                    nc.vector.memset(e1[0:32, :], 0.0)
                    nc.vector.memset(e1[32:33, :], 1.0)
                    be1.append(e1)
                    beps.append(brp.tile([16, BP], f32, name=f"beps{r}", tag=f"beps{r}"))
                nc.sync.dma_start(out=bhz[0][16:32, :], in_=hseq[0:16, 0:BP])
                nc.sync.dma_start(out=beps[0][:, :], in_=eps_in[0, :, :])

                HB = BP // 2
                chs = [slice(0, HB), slice(HB, BP)]
                def b_stage1(t, ci, cs):
                    r = t % RB
                    p1 = psB.tile(
                        [16, HB], f32, name=f"p1{ci}", tag=f"p1{ci}", bufs=1
                    )
                    nc.tensor.matmul(
                        p1[:, :],
                        _bc(wtm["comb1_lhsT"][:, :]),
                        _bc(bhz[r][:, cs]),
                        start=True,
                        stop=True,
                    )
                    nc.scalar.activation(bt1[r][0:16, cs], p1[:, :], AF.Tanh)

                def b_stage2(t, ci, cs):
                    r = t % RB
                    p2 = psB.tile(
                        [16, HB], f32, name=f"p2{ci}", tag=f"p2{ci}", bufs=1
                    )
                    nc.tensor.matmul(
                        p2[:, :],
                        _bc(wtm["wp_lhsT"][:, :]),
                        _bc(bt1[r][:, cs]),
                        start=True,
                        stop=True,
                    )
                    nc.scalar.activation(be1[r][0:16, cs], p2[:, :], AF.Tanh)

                def b_stage3(t, ci, cs):
                    r = t % RB
                    rn = (t + 1) % RB
                    p3 = psB.tile(
                        [48, HB], f32, name=f"p3{ci}", tag=f"p3{ci}", bufs=2
                    )
                    nc.tensor.matmul(
                        p3[:, :],
                        _bc(wtm["enc2_lhsT"][:, :]),
                        _bc(be1[r][:, cs]),
                        start=True,
                        stop=True,
                    )
                    estg = sB.tile([48, HB], f32, name=f"estg{ci}", tag=f"estg{ci}")
                    nc.vector.tensor_copy(estg[:, :], p3[:, :])
                    E = sB.tile([16, HB], f32, name=f"E{ci}", tag=f"E{ci}")
                    nc.scalar.activation(E[:, :], p3[32:48, :], AF.Exp, scale=0.5)
                    tt = sB.tile([16, HB], f32, name=f"tt{ci}", tag=f"tt{ci}")
                    nc.vector.tensor_mul(tt[:, :], beps[r][:, cs], E[:, :])
                    nc.vector.tensor_add(bhz[rn][0:16, cs], tt[:, :], p3[0:16, :])
                    nc.sync.dma_start(out=out_e[t, 0:16, cs], in_=estg[0:16, :])
                    nc.sync.dma_start(out=out_e[t, 16:32, cs], in_=estg[32:48, :])

                for t in range(T):
                    rn = (t + 1) % RB
                    b_stage1(t, 0, chs[0])
                    b_stage1(t, 1, chs[1])
                    b_stage2(t, 0, chs[0])
                    b_stage2(t, 1, chs[1])
                    b_stage3(t, 0, chs[0])
                    b_stage3(t, 1, chs[1])
                    g, cc = t % 8, t // 8
                    zdst = zpk[16 * g : 16 * g + 16, BP * cc : BP * cc + BP]
                    nc.sync.dma_start(out=zdst, in_=bhz[rn][0:16, :])
                    if t < T - 1:
                        tn = t + 1
                        gn, ccn = tn % 8, tn // 8
                        hsrc = hseq[16 * gn : 16 * gn + 16, BP * ccn : BP * ccn + BP]
                        nc.sync.dma_start(out=bhz[rn][16:32, :], in_=hsrc)
                        nc.sync.dma_start(out=beps[rn][:, :], in_=eps_in[tn, :, :])

            # ============== Phase C: decoder + transition (bulk) ==============
            with (
                tc.tile_pool(name="cpsum", bufs=2, space="PSUM") as psC,
                tc.tile_pool(name="cpsum2", bufs=2, space="PSUM") as psC2,
                tc.tile_pool(name="csb", bufs=2) as sC,
            ):
                for m in range(NM):
                    cols = slice(512 * m, 512 * m + 512)
                    # ---- decoder ----
                    pd1 = psC.tile([128, 512], f32, name="pd1", tag="pd1")
                    nc.tensor.matmul(
                        pd1[:, :],
                        _bc(wtm["dec1bd"][:, :]),
                        _bc(zpk[:, cols]),
                        start=True,
                        stop=True,
                    )
                    d1 = sC.tile([128, 512], mmdt, name="d1", tag="d1")
                    nc.scalar.activation(
                        d1[:, :], pd1[:, :], AF.Tanh, bias=wt["dec1b"][:, :]
                    )
                    pd2 = psC2.tile([16, 512], f32, name="pd2", tag="pd2")
                    nc.tensor.matmul(
                        pd2[:, :],
                        _bc(wtm["dec2bd"][:, :]),
                        _bc(d1[:, :]),
                        start=True,
                        stop=True,
                    )
                    dstg = sC.tile([16, 512], f32, name="dstg", tag="dstg")
                    nc.scalar.activation(
                        dstg[:, :], pd2[:, :], AF.Identity, bias=wt["dec2b"][:, :]
                    )
                    nc.sync.dma_start(out=out_d[m, :, :], in_=dstg[:, :])
                    # ---- transition ----
                    ptr = psC.tile([128, 512], f32, name="ptr", tag="ptr")
                    nc.tensor.matmul(
                        ptr[:, :],
                        _bc(wtm["tr1bd"][:, :]),
                        _bc(zpk[:, cols]),
                        start=True,
                        stop=False,
                        skip_group_check=True,
                    )
                    for jj in range(2):
                        cj = 2 * m + jj
                        if cj == 0:
                            prhs = zzero[:, :]
                        else:
                            # PE requires operand base partition in {0,32,64};
                            # stage the group-7 slice down to partition 0.
                            zps = sC.tile([16, BP], mmdt, name="zps", tag="zps", bufs=3)
                            nc.sync.dma_start(
                                out=zps[:, :],
                                in_=zpk[112:128, BP * (cj - 1) : BP * cj],
                            )
                            prhs = zps[:, :]
                        nc.tensor.matmul(
                            ptr[0:16, 256 * jj : 256 * jj + 256],
                            _bc(wtm["tr1g0"][:, :]),
                            _bc(prhs),
                            start=False,
                            stop=(jj == 1),
                            skip_group_check=True,
                        )
                    d1t = sC.tile([128, 512], mmdt, name="d1t", tag="d1t")
                    nc.scalar.activation(
                        d1t[:, :], ptr[:, :], AF.Tanh, bias=wt["tr1b"][:, :]
                    )
                    for half in range(2):
                        pt2 = psC2.tile([128, 512], f32, name="pt2", tag="pt2")
                        nc.tensor.matmul(
                            pt2[:, :],
                            _bc(wtm["tr2bd"][64 * half : 64 * half + 64, :]),
                            _bc(d1t[64 * half : 64 * half + 64, :]),
                            start=True,
                            stop=True,
                        )
                        tstg = sC.tile([128, 512], f32, name="tstg", tag="tstg")
                        nc.scalar.activation(
                            tstg[:, :], pt2[:, :], AF.Identity, bias=wt["tr2b"][:, :]
                        )
                        nc.sync.dma_start(out=out_t[m, half, :, :], in_=tstg[:, :])

    nc.finalize()
    return nc


def decode_outputs(res, T=T_LEN):
    """Unpack one core's device outputs into (T, BP, 66)."""
    NM = T // 16
    oe = res["out_e"]  # (T, 32, BP)
    mu_z = oe[:, 0:16, :].transpose(0, 2, 1)
    lv_z = oe[:, 16:32, :].transpose(0, 2, 1)
    od = res["out_d"].reshape(NM, 8, 2, 2, BP)  # (m, g, f, j, b)
    od = od.transpose(0, 3, 1, 2, 4).reshape(T, 2, BP)  # t = (m, j, g)
    mu_x = od[:, 0, :][:, :, None]
    lv_x = od[:, 1, :][:, :, None]
    ot = res["out_t"].reshape(NM, 2, 4, 32, 2, BP)  # (m, half, h, f, j, b)
    ot = ot.transpose(0, 4, 1, 2, 3, 5).reshape(T, 32, BP)  # t = (m, j, half, h)
    mu_t = ot[:, 0:16, :].transpose(0, 2, 1)
    lv_t = ot[:, 16:32, :].transpose(0, 2, 1)
    return np.concatenate([mu_x, lv_x, mu_z, lv_z, mu_t, lv_t], axis=-1)


def make_in_maps(inputs, T=T_LEN, mm_dtype=None):
    if mm_dtype is None:
        mm_dtype = MM_DTYPE
    W = _prep_weights(inputs)
    x = np.asarray(inputs["x"])  # (T, B, 1)
    eps = np.asarray(inputs["eps_z"])  # (T, B, Z)
    if mm_dtype == "bf16":
        import ml_dtypes

        xdt = ml_dtypes.bfloat16
    else:
        xdt = np.float32
    in_maps = []
    for c in range(N_CORES):
        sl = slice(BP * c, BP * c + BP)
        m = dict(W)
        m["x_r"] = np.ascontiguousarray(x[:T, sl, 0], dtype=xdt)
        m["eps_fm"] = _f(eps[:T, sl, :].transpose(0, 2, 1))
        in_maps.append(m)
    return in_maps


def kernel(**inputs):
    from concourse.bass_utils import run_bass_kernel_spmd

    key = (T_LEN, MM_DTYPE)
    if key not in _CACHE:
        _CACHE[key] = build_nc(T_LEN, MM_DTYPE)
    nc = _CACHE[key]
    in_maps = make_in_maps(inputs, T_LEN, MM_DTYPE)
    res = run_bass_kernel_spmd(nc, in_maps, list(range(N_CORES)))
    outs = [decode_outputs(r, T_LEN) for r in res.results]
    return np.concatenate(outs, axis=1)

